# revision 28
# baseline (speedup 1.0000x reference)
"""Trainium2 Bass kernel for nn_Decoder_42417097016016 (DCTTS-style decoder).

Sharding: pure data parallel over batch. B=16 samples -> 8 NeuronCores x 2
samples each; all weights replicated per core.

Layout: activations live on-chip as (channels, time) so every conv1x1 /
causal conv is a PE matmul with channels on partitions.  Causal convs with
dilation d are 3 accumulating matmuls per output tile with column shifts
(0, d, 2d) - left zero-padding falls out of the shifted PSUM accumulation.

The attention block computes scores (t,s), softmax along free dim (ACT Exp
with accum_out row sums), writes the normalized attn output directly, and
PE-transposes it to (s,t) for the context matmul.  mel and done share one
final matmul by concatenating fc_w as a 401st output column (padded to 402).

Matmul precision knob KBENCH_MM: "f16" (default, full PE rate), "f32r"
(reduced-precision fp32, half rate, needs even matmul geometry -> shifted x2
copies for odd-dilation taps), "f32" (exact, quarter rate).
"""

import math
import os
import sys

import numpy as np

for _p in ("/opt/trn_rl_repo", "/root/.axon_site/_ro/trn_rl_repo"):
    if os.path.isdir(_p) and _p not in sys.path:
        sys.path.append(_p)

import concourse.bass as bass
import concourse.tile as tile
from concourse import bacc, mybir
from concourse.bass_utils import run_bass_kernel_spmd

AF = mybir.ActivationFunctionType
ALU = mybir.AluOpType
AX = mybir.AxisListType
F32 = mybir.dt.float32
F32R = mybir.dt.float32r
F16 = mybir.dt.float16

N_CORES = 8
B, T, TE, D, F = 16, 512, 256, 256, 400
BS = B // N_CORES  # samples per core
DIL = [1, 3, 9, 27, 1, 3, 9, 27, 3, 3] + [1, 3, 9, 27, 1, 1]
L = len(DIL)  # 16 highway layers (10 encoder + 6 decoder)
SQ2 = math.sqrt(0.5)

MM_DT = os.environ.get("KBENCH_MM", "f16")

# bias table column assignments
COL_ENC0, COL_ENC1, COL_ENC2 = 0, 2, 4
COL_QB, COL_OB = 6, 8
COL_DEC0, COL_DEC1, COL_DEC2, COL_DEC3 = 10, 12, 14, 16
NB = 18

LAST_EXEC_NS = None
_BUILD_CACHE = {}


def _mm(nc, out, lhsT, rhs, **kw):
    nc.tensor.matmul(out, lhsT, rhs, **kw)


def _build():
    """Build the per-core Bass program (same program on all 8 cores)."""
    from concourse.masks import make_identity

    nc = bacc.Bacc("TRN2", target_bir_lowering=False, debug=False)
    dt = F32
    dtm = {"f16": F16, "f32r": F32R, "f32": F32}[MM_DT]
    use_x2 = MM_DT == "f32r"  # f32r matmuls need even offsets/N

    # ---- DRAM I/O (per-core shard shapes) ----
    # x0: (s, p, c, t) pre-chunked+padded on host -> one DMA per sample
    d_x0 = nc.dram_tensor("x0", [BS, 128, 4, T], dtm, kind="ExternalInput").ap()
    d_hw_w = nc.dram_tensor("hw_w", [L, 128, 2, 3, 4, 128], dtm, kind="ExternalInput").ap()
    # wenc0p: enc_w0^T padded 400->512 rows, chunked (128, 4, 256)
    d_wenc0 = nc.dram_tensor("wenc0p", [128, 4, D], dtm, kind="ExternalInput").ap()
    # pack1: [enc1 c0|c1, enc2 c0|c1] as (128, 4*256)
    d_pack1 = nc.dram_tensor("pack1", [128, 4 * D], dtm, kind="ExternalInput").ap()
    # bias2: [bias_tbl (18) | hw_b (L*4)] fp32
    d_bias2 = nc.dram_tensor("bias2", [128, NB + L * 4], dt, kind="ExternalInput").ap()
    # wpack: all attention/decoder weights + keys/values + ones/blast rows
    WQ_O, WO_O = 0, 512
    WD0_O, WD1_O, WD2_O, WD3_O = 1024, 2048, 2560, 3072
    WLAST_O = 3584
    KT_O = WLAST_O + 2 * (F + 2)          # 4388
    V_O = KT_O + 4 * TE                   # 5412
    ONES_O = V_O + 4 * D                  # 6436
    BLAST_O = ONES_O + 128                # 6564
    WPACK_COLS = BLAST_O + (F + 2)        # 6966
    d_wpack = nc.dram_tensor("wpack", [128, WPACK_COLS], dtm, kind="ExternalInput").ap()

    d_mel = nc.dram_tensor("mel", [BS, T, F], dt, kind="ExternalOutput").ap()
    d_attn = nc.dram_tensor("attn", [BS, T, TE], dt, kind="ExternalOutput").ap()
    d_done = nc.dram_tensor("done", [BS, T, 1], dt, kind="ExternalOutput").ap()

    with tile.TileContext(nc) as tc:
        with (
            tc.tile_pool(name="const", bufs=1) as const,
            tc.tile_pool(name="xpool", bufs=1) as xpool,
            tc.tile_pool(name="persist", bufs=1) as persist,
            tc.tile_pool(name="wstream", bufs=1) as wstream,
            tc.tile_pool(name="temp", bufs=1) as temp,
            tc.tile_pool(name="psum", bufs=1, space="PSUM") as psum,
        ):
            # ---------- startup-critical loads first (DMA queue is FIFO) ----
            wenc0_sb = const.tile([128, 4, D], dtm, name="wenc0_sb")
            nc.sync.dma_start(wenc0_sb, d_wenc0)
            w_enc0_sb = [wenc0_sb[:, c, :] for c in range(4)]
            xin = {}
            for s in range(BS):
                xt = temp.tile([128, 4, T], dtm, tag=f"xin_{s}", name=f"xin_{s}")
                nc.sync.dma_start(xt, d_x0[s])
                xin[s] = [xt[:, c, :] for c in range(4)]
            pack1_sb = const.tile([128, 4 * D], dtm, name="pack1_sb")
            nc.sync.dma_start(pack1_sb, d_pack1)
            w_enc1_sb = [pack1_sb[:, 0:D], pack1_sb[:, D : 2 * D]]
            w_enc2_sb = [pack1_sb[:, 2 * D : 3 * D], pack1_sb[:, 3 * D : 4 * D]]
            bias2_sb = const.tile([128, NB + L * 4], dt, name="bias2_sb")
            nc.sync.dma_start(bias2_sb, d_bias2)
            bias_sb = bias2_sb[:, 0:NB]

            def hwb_ap(l, j):
                c = NB + l * 4 + j
                return bias2_sb[:, c : c + 1]

            ident = const.tile([128, 128], dt, name="ident")
            make_identity(nc, ident)
            stat_sb = const.tile([128, 32], F32, name="stat_sb")
            # HAM warm-up: keep the PE busy during the input DMA wait so the
            # clock gate is already at 8/8 when real matmuls arrive.
            warm_ps = psum.tile([128, 128], F32, tag="bank7", name="warm_ps")
            for _w in range(24):
                nc.tensor.matmul(warm_ps, ident, ident, is_transpose=True,
                                 start=True, stop=True, skip_group_check=True)

            # helper: one conv1x1 block (256 outputs) for all samples.
            # Epilogue on DVE: out = relu?(psum + bias)
            def conv_block(w_tiles, rhs_per_s, relu, bias_col, banks, out_pool,
                           tag_fn, bufs=1, uid=""):
                outs = {s: [None, None] for s in range(BS)}
                pss = {}
                for mt in range(2):
                    for s in range(BS):
                        ps = psum.tile(
                            [128, T], F32,
                            tag=f"bank{banks[s * 2 + mt]}",
                            name=f"ps_{uid}_{s}_{mt}",
                        )
                        nk = len(w_tiles)
                        for c in range(nk):
                            _mm(nc, ps, w_tiles[c][:, mt * 128 : (mt + 1) * 128],
                                rhs_per_s[s][c], start=(c == 0), stop=(c == nk - 1))
                        pss[(s, mt)] = ps
                for mt in range(2):
                    for s in range(BS):
                        ot = out_pool.tile(
                            [128, T], dtm,
                            tag=tag_fn(s, mt), bufs=bufs,
                            name=f"{uid}_{s}_{mt}",
                        )
                        b_ap = bias_sb[:, bias_col + mt : bias_col + mt + 1]
                        if mt == 0:  # split epilogues across DVE and ACT
                            if relu:
                                nc.vector.tensor_scalar(ot, pss[(s, mt)], b_ap, 0.0,
                                                        op0=ALU.add, op1=ALU.max)
                            else:
                                nc.vector.tensor_scalar_add(ot, pss[(s, mt)], b_ap)
                        else:
                            nc.scalar.activation(
                                ot, pss[(s, mt)], AF.Relu if relu else AF.Identity,
                                bias=b_ap, scale=1.0)
                        outs[s][mt] = ot
                return outs

            xtag = lambda s, mt: f"x_{s}_{mt}"
            gatag = lambda s, mt: f"ga_{s}_{mt}"
            gbtag = lambda s, mt: f"gb_{s}_{mt}"

            # ---------- encoder head ----------
            h1 = conv_block(w_enc0_sb, xin, True, COL_ENC0, [0, 1, 4, 5], temp, gatag, uid="h1")
            h2 = conv_block(w_enc1_sb, h1, True, COL_ENC1, [2, 3, 6, 7], temp, gbtag, uid="h2")
            xs = conv_block(w_enc2_sb, h2, False, COL_ENC2, [0, 1, 4, 5], xpool, xtag,
                            bufs=4, uid="xe")

            # remaining const loads are deferred (emitted mid-encoder so the
            # first highway-layer weight DMAs aren't stuck behind them)
            misc = {}

            def load_misc_consts():
                wp = const.tile([128, WPACK_COLS], dtm, name="wpack_sb")
                nc.sync.dma_start(wp, d_wpack)
                misc["w_q"] = [wp[:, WQ_O : WQ_O + D], wp[:, WQ_O + D : WQ_O + 2 * D]]
                misc["w_o"] = [wp[:, WO_O : WO_O + D], wp[:, WO_O + D : WO_O + 2 * D]]
                misc["w_dec0"] = [wp[:, WD0_O + c * D : WD0_O + (c + 1) * D] for c in range(4)]
                misc["w_dec1"] = [wp[:, WD1_O : WD1_O + D], wp[:, WD1_O + D : WD1_O + 2 * D]]
                misc["w_dec2"] = [wp[:, WD2_O : WD2_O + D], wp[:, WD2_O + D : WD2_O + 2 * D]]
                misc["w_dec3"] = [wp[:, WD3_O : WD3_O + D], wp[:, WD3_O + D : WD3_O + 2 * D]]
                misc["w_last"] = [wp[:, WLAST_O : WLAST_O + F + 2],
                                  wp[:, WLAST_O + F + 2 : WLAST_O + 2 * (F + 2)]]
                misc["ones"] = wp[0:2, ONES_O : ONES_O + 128]
                misc["blast"] = wp[0:2, BLAST_O : BLAST_O + F + 2]
                kT_sb, v_sb = {}, {}
                for s in range(BS):
                    for c in range(2):
                        i = s * 2 + c
                        kT_sb[(s, c)] = wp[:, KT_O + i * TE : KT_O + (i + 1) * TE]
                        v_sb[(s, c)] = wp[:, V_O + i * D : V_O + (i + 1) * D]
                misc["kT"] = kT_sb
                misc["v"] = v_sb

            # ---------- highway stack ----------
            def make_x2(xs_cur, uid):
                # right-shifted copy (col0 = 0): makes the odd-dilation middle
                # conv tap even-aligned for the f32r matmul mode.
                out = {s: [None, None] for s in range(BS)}
                for c in range(2):
                    for s in range(BS):
                        x2 = xpool.tile([128, T], dtm, tag=f"x2_{s}_{c}", bufs=2,
                                        name=f"x2_{uid}_{s}_{c}")
                        nc.gpsimd.tensor_scalar_mul(x2[:, 0:1], xs_cur[s][c][:, 0:1], 0.0)
                        nc.gpsimd.tensor_copy(x2[:, 1:T], xs_cur[s][c][:, 0 : T - 1])
                        out[s][c] = x2
                return out

            def highway_layers(l_lo, l_hi, x2s):
                nonlocal xs
                for l in range(l_lo, l_hi):
                    dil = DIL[l]
                    wt = wstream.tile([128, 2, 3, 4, 128], dtm, tag="hww", bufs=3,
                                      name=f"hw_w_{l}")
                    nc.sync.dma_start(wt, d_hw_w[l])
                    last_enc = l == 9
                    ps_all = {}
                    for mt in range(4):
                        for s in range(BS):
                            ps_all[(mt, s)] = psum.tile(
                                [128, T], F32, tag=f"bank{mt * 2 + s}",
                                name=f"hwps_{l}_{mt}_{s}",
                            )
                    # kc-major: ALL kc0 taps (24 matmuls) before any kc1 tap,
                    # giving the previous layer's x'[1] epilogue a ~5us runway.
                    # mt order (2,0,3,1): chunk-0 gate+input banks finish first
                    # so its epilogue chain overlaps the remaining matmuls.
                    seen = {}
                    for kc in range(2):
                        for mt in (2, 0, 3, 1):
                            for k in (2, 1, 0):
                                first = kc == 0 and k == 2
                                last = kc == 1 and k == 0
                                for s in range(BS):
                                    ps = ps_all[(mt, s)]
                                    if k == 2:
                                        _mm(nc, ps, wt[:, kc, k, mt, :], xs[s][kc],
                                            start=first, stop=last)
                                    elif k == 1:
                                        if use_x2:
                                            off = dil - 1
                                            _mm(nc, ps[:, off:T], wt[:, kc, k, mt, :],
                                                x2s[s][kc][:, 0 : T - off],
                                                start=first, stop=last)
                                        else:
                                            _mm(nc, ps[:, dil:T], wt[:, kc, k, mt, :],
                                                xs[s][kc][:, 0 : T - dil],
                                                start=first, stop=last)
                                    else:
                                        sh = 2 * dil
                                        _mm(nc, ps[:, sh:T], wt[:, kc, k, mt, :],
                                            xs[s][kc][:, 0 : T - sh],
                                            start=first, stop=last)
                    # epilogue: x' = x + sigmoid(g) * (a - x)
                    # per chunk c: g = bank (c+2), a = bank c
                    newxs = {s: [None, None] for s in range(BS)}
                    for c in range(2):
                        tgs = {}
                        for s in range(BS):
                            tg = temp.tile([128, T], dtm, tag=f"tg_{s}_{c}", bufs=3,
                                           name=f"tg_{l}_{s}_{c}")
                            nc.scalar.activation(
                                tg, ps_all[(c + 2, s)], AF.Sigmoid,
                                bias=hwb_ap(l, c + 2), scale=1.0)
                            tgs[s] = tg
                        tmps = {}
                        for s in range(BS):
                            tmp = temp.tile([128, T], dtm, tag=f"tmp_{s}_{c}", bufs=3,
                                            name=f"tmp_{l}_{s}_{c}")
                            nc.vector.scalar_tensor_tensor(
                                tmp, ps_all[(c, s)], hwb_ap(l, c),
                                xs[s][c], op0=ALU.add, op1=ALU.subtract)
                            tmps[s] = tmp
                        for s in range(BS):
                            nc.vector.tensor_mul(tmps[s], tgs[s], tmps[s])
                        for s in range(BS):
                            if last_enc:
                                xn = persist.tile([128, T], dtm, tag=f"q_{s}_{c}",
                                                  name=f"Q_{s}_{c}")
                            else:
                                xn = xpool.tile([128, T], dtm, tag=f"x_{s}_{c}", bufs=4,
                                                name=f"x_{l + 1}_{s}_{c}")
                            nc.vector.tensor_add(xn, tmps[s], xs[s][c])
                            newxs[s][c] = xn
                    xs = newxs
                    if use_x2 and l + 1 < l_hi:
                        x2s = make_x2(xs, f"l{l + 1}")
                    if l == 1:
                        load_misc_consts()

            # ---------- encoder highway ----------
            highway_layers(0, 10, make_x2(xs, "e0") if use_x2 else None)
            w_q_sb, w_o_sb = misc["w_q"], misc["w_o"]
            w_dec0_sb, w_dec1_sb = misc["w_dec0"], misc["w_dec1"]
            w_dec2_sb, w_dec3_sb = misc["w_dec2"], misc["w_dec3"]
            w_last_sb, ones_row, blast_sb = misc["w_last"], misc["ones"], misc["blast"]
            kT_sb, v_sb = misc["kT"], misc["v"]
            Qs = xs  # persisted encoder output (D, T) tiles

            # ---------- attention ----------
            Rqs = {s: [] for s in range(BS)}
            q_sb = {s: [] for s in range(BS)}
            for mt in range(2):
                for s in range(BS):
                    ps = psum.tile([128, T], F32, tag=f"bank{s * 4 + mt}",
                                   name=f"qps_{s}_{mt}")
                    for kc in range(2):
                        _mm(nc, ps, w_q_sb[kc][:, mt * 128 : (mt + 1) * 128], Qs[s][kc],
                            start=(kc == 0), stop=(kc == 1))
                    qt = temp.tile([128, T], dtm, tag=f"ga_{s}_{mt}", name=f"q_{s}_{mt}")
                    nc.vector.tensor_scalar_add(
                        qt, ps, bias_sb[:, COL_QB + mt : COL_QB + mt + 1])
                    q_sb[s].append(qt)

            # softmax without max-subtraction: scores are O(10), exp is safe in
            # fp32 and softmax is shift-invariant, so this matches the reference.
            # Stage-major across samples so the PE always has the other
            # sample's matmuls while one sample's softmax chain runs.
            at_tiles, psts, aT = {}, {}, {s: [] for s in range(BS)}
            for s in range(BS):
                at_tiles[s] = temp.tile([128, 4, TE], dt, tag=f"att_{s}",
                                        name=f"att_{s}")
                psts[s] = [psum.tile([128, T], F32, tag=f"bank{s * 4 + 2 + sc}",
                                     name=f"tps_{s}_{sc}") for sc in range(2)]
            for s in range(BS):
                for tt in range(4):
                    ps = psum.tile([128, TE], F32, tag=f"bank{s * 4 + (tt % 2)}",
                                   name=f"sps_{s}_{tt}")
                    for dc in range(2):
                        _mm(nc, ps, q_sb[s][dc][:, tt * 128 : (tt + 1) * 128],
                            kT_sb[(s, dc)], start=(dc == 0), stop=(dc == 1))
                    st = stat_sb[:, (s * 4 + tt) * 4 : (s * 4 + tt) * 4 + 4]
                    at = at_tiles[s][:, tt, :]
                    nc.scalar.activation(at, ps, AF.Exp, accum_out=st[:, 1:2])
                    nc.vector.reciprocal(st[:, 2:3], st[:, 1:2])
                    nc.vector.tensor_scalar_mul(at, at, st[:, 2:3])
            # dec conv0, part 1: Q-chunk matmuls are attention-independent ->
            # emit them here so the PE stays busy (and HAM warm) while the
            # softmax chains run.  Result bounces to SBUF; part 2 adds it back.
            qcontrib = {}
            for s in range(BS):
                for mt in range(2):
                    ps = psum.tile([128, T], F32, tag=f"bank{s * 4 + mt}",
                                   name=f"ps_xd0a_{s}_{mt}")
                    for c in range(2):
                        _mm(nc, ps, w_dec0_sb[c][:, mt * 128 : (mt + 1) * 128],
                            Qs[s][c], start=(c == 0), stop=(c == 1))
                    qc = temp.tile([128, T], F32, tag=f"qc_{s}_{mt}",
                                   name=f"qc_{s}_{mt}")
                    if mt == 0:
                        nc.vector.tensor_copy(qc, ps)
                    else:
                        nc.scalar.copy(qc, ps)
                    qcontrib[(s, mt)] = qc
            for s in range(BS):
                for tt in range(4):
                    at = at_tiles[s][:, tt, :]
                    for sc in range(2):
                        nc.tensor.matmul(
                            psts[s][sc][:, tt * 128 : (tt + 1) * 128],
                            at[:, sc * 128 : (sc + 1) * 128],
                            ident, is_transpose=True, start=True, stop=True,
                            skip_group_check=True)
                nc.sync.dma_start(
                    d_attn[s].rearrange("(tt p) e -> p tt e", p=128), at_tiles[s])
            for s in range(BS):
                for sc in range(2):
                    a2 = temp.tile([128, T], dtm, tag=f"gb_{s}_{sc}", name=f"aT_{s}_{sc}")
                    nc.vector.tensor_copy(a2, psts[s][sc])
                    aT[s].append(a2)

            ctx_sb = {s: [] for s in range(BS)}
            for s in range(BS):
                for dc in range(2):
                    ps = psum.tile([128, T], F32, tag=f"bank{s * 4 + dc}",
                                   name=f"cps_{s}_{dc}")
                    for sc in range(2):
                        _mm(nc, ps, v_sb[(s, sc)][:, dc * 128 : (dc + 1) * 128],
                            aT[s][sc], start=(sc == 0), stop=(sc == 1))
                    ct = temp.tile([128, T], dtm, tag=f"ga_{s}_{dc}", name=f"ctx_{s}_{dc}")
                    nc.vector.tensor_copy(ct, ps)
                    ctx_sb[s].append(ct)

            for mt in range(2):
                for s in range(BS):
                    ps = psum.tile([128, T], F32, tag=f"bank{s * 4 + mt}",
                                   name=f"ops_{s}_{mt}")
                    for dc in range(2):
                        _mm(nc, ps, w_o_sb[dc][:, mt * 128 : (mt + 1) * 128],
                            ctx_sb[s][dc], start=(dc == 0), stop=(dc == 1))
                    tmpo = temp.tile([128, T], dt, tag=f"gb_{s}_{mt}",
                                     name=f"tmpo_{s}_{mt}")
                    nc.vector.tensor_scalar_add(
                        tmpo, ps, bias_sb[:, COL_OB + mt : COL_OB + mt + 1])
                    rq = persist.tile([128, T], dtm, tag=f"rq_{s}_{mt}",
                                      name=f"rq_{s}_{mt}")
                    # Rq = sqrt(.5)*query + out_proj  [scales folded into w_o/b_o]
                    nc.vector.scalar_tensor_tensor(
                        rq, Qs[s][mt], SQ2, tmpo, op0=ALU.mult, op1=ALU.add)
                    Rqs[s].append(rq)

            # ---------- decoder (conv0 part 2: Rq chunks + fused epilogue) ----
            xs = {s: [None, None] for s in range(BS)}
            d0ps = {}
            for mt in range(2):
                for s in range(BS):
                    ps = psum.tile([128, T], F32, tag=f"bank{[2, 3, 6, 7][s * 2 + mt]}",
                                   name=f"ps_xd0b_{s}_{mt}")
                    for c in range(2):
                        _mm(nc, ps, w_dec0_sb[2 + c][:, mt * 128 : (mt + 1) * 128],
                            Rqs[s][c], start=(c == 0), stop=(c == 1))
                    d0ps[(s, mt)] = ps
            for mt in range(2):
                for s in range(BS):
                    ot = xpool.tile([128, T], dtm, tag=f"x_{s}_{mt}", bufs=4,
                                    name=f"xd0_{s}_{mt}")
                    b_ap = bias_sb[:, COL_DEC0 + mt : COL_DEC0 + mt + 1]
                    nc.vector.scalar_tensor_tensor(
                        ot, d0ps[(s, mt)], b_ap, qcontrib[(s, mt)],
                        op0=ALU.add, op1=ALU.add)
                    xs[s][mt] = ot
            highway_layers(10, 16, make_x2(xs, "d0") if use_x2 else None)
            xs = conv_block(w_dec1_sb, xs, True, COL_DEC1, [0, 1, 4, 5], xpool, xtag,
                            bufs=4, uid="xd1")
            xs = conv_block(w_dec2_sb, xs, True, COL_DEC2, [2, 3, 6, 7], xpool, xtag,
                            bufs=4, uid="xd2")
            xs = conv_block(w_dec3_sb, xs, True, COL_DEC3, [0, 1, 4, 5], xpool, xtag,
                            bufs=4, uid="xd3")

            # ---------- final: mel (per-tt sigmoid conv) + done ((1,T) row) ----
            for s in range(BS):
                # done = sigmoid(fc . x) computed as a single-row matmul so the
                # output DMA is one contiguous 2KB write (not a 512-desc scatter)
                psd = psum.tile([1, T], F32, tag=f"bank{s * 4 + 3}", name=f"dps_{s}")
                for dc in range(2):
                    _mm(nc, psd, w_last_sb[dc][:, F : F + 1], xs[s][dc],
                        start=(dc == 0), stop=(dc == 1))
                dn = temp.tile([1, T], dt, tag=f"done_{s}", name=f"done_{s}")
                nc.scalar.activation(dn, psd, AF.Sigmoid, scale=1.0,
                                     bias=blast_sb[0:1, F : F + 1])
                nc.sync.dma_start(d_done[s].rearrange("t o -> o t"), dn)
            for s in range(BS):
                fo = temp.tile([128, 4, F + 2], dt, tag=f"fin_{s}", name=f"fin_{s}")
                for tt in range(4):
                    ps = psum.tile([128, F + 2], F32, tag=f"bank{s * 4 + tt}",
                                   name=f"fps_{s}_{tt}")
                    for dc in range(2):
                        _mm(nc, ps, xs[s][dc][:, tt * 128 : (tt + 1) * 128],
                            w_last_sb[dc], start=(dc == 0), stop=False)
                    _mm(nc, ps, ones_row, blast_sb, start=False, stop=True)
                    nc.scalar.activation(fo[:, tt, :], ps, AF.Sigmoid, scale=1.0)
                    nc.sync.dma_start(d_mel[s, tt * 128 : (tt + 1) * 128, :],
                                      fo[:, tt, 0:F])

    nc.compile()
    return nc


def _prep_host(inputs):
    """Host-side packing: transposes, chunking, and packed const blocks."""
    f32 = np.float32
    mm_np = np.float16 if MM_DT == "f16" else np.float32

    def npm(a):
        return np.ascontiguousarray(np.asarray(a, dtype=f32)).astype(mm_np)

    # x0: (B, T, F) -> pad F to 512 -> (B, 128, 4, T)
    x_t = np.zeros((B, 512, T), f32)
    x_t[:, :F, :] = np.asarray(inputs["inputs"], f32).transpose(0, 2, 1)
    x0 = npm(x_t.reshape(B, 4, 128, T).transpose(0, 2, 1, 3))

    keysT = np.asarray(inputs["keys"], f32).transpose(0, 2, 1)  # (B, D, TE)
    values = np.asarray(inputs["values"], f32)  # (B, TE, D)

    w_all = np.concatenate([np.asarray(inputs["enc_hw_w"]),
                            np.asarray(inputs["dec_hw_w"])], axis=0)  # (16, 512, 256, 3)
    wt = w_all.transpose(0, 2, 1, 3)            # (L, ci, co, k)
    wt = wt.reshape(L, 2, 128, 4, 128, 3)       # (L, kc, p, mt, f, k)
    hw_w = npm(wt.transpose(0, 2, 1, 5, 3, 4))  # (L, 128, kc, k, mt, f)

    def t2(w):  # (O, I, 1) -> (I, O) fp32
        return np.asarray(w, f32)[:, :, 0].T

    # wenc0p: (400, 256) -> pad rows to 512 -> (128, 4, 256)
    we0 = np.zeros((512, D), f32)
    we0[:F] = t2(inputs["enc_w0"])
    wenc0p = npm(we0.reshape(4, 128, D).transpose(1, 0, 2))

    def chunks(w):  # (rows, cols) -> list of (128, cols)
        return [w[c * 128 : (c + 1) * 128] for c in range(w.shape[0] // 128)]

    pack1 = npm(np.concatenate(
        chunks(t2(inputs["enc_w1"])) + chunks(t2(inputs["enc_w2"])), axis=1))

    # wpack: [wq | wo | wdec0 | wdec1 | wdec2 | wdec3 | wlast | kT | v | ones | blast]
    w_q = np.asarray(inputs["attn_q_w"], f32).T
    w_o = np.asarray(inputs["attn_o_w"], f32).T * (math.sqrt(TE) * SQ2)
    w_last = np.concatenate(
        [np.asarray(inputs["last_w"], f32)[:, :, 0].T,
         np.asarray(inputs["fc_w"], f32).T,
         np.zeros((D, 1), f32)], axis=1)  # (256, 402)
    wd0 = chunks(t2(inputs["dec_w0"]))
    wd0 = [wd0[2], wd0[3], wd0[0], wd0[1]]  # ci chunks reordered: Q first, Rq last
    blocks = (chunks(w_q) + chunks(w_o) + wd0
              + chunks(t2(inputs["dec_w1"])) + chunks(t2(inputs["dec_w2"]))
              + chunks(t2(inputs["dec_w3"])) + chunks(w_last))
    # keys/values are per-core; build the shared prefix once
    prefix = np.concatenate(blocks, axis=1)  # (128, 4388)
    ones_blk = np.zeros((128, 128), f32)
    ones_blk[0, :] = 1.0
    blast_blk = np.zeros((128, F + 2), f32)
    blast_blk[0, :F] = np.asarray(inputs["last_b"], f32)
    blast_blk[0, F] = np.asarray(inputs["fc_b"], f32)[0]

    b_all = np.concatenate([np.asarray(inputs["enc_hw_b"]),
                            np.asarray(inputs["dec_hw_b"])], axis=0)  # (16, 512)
    hw_b = np.asarray(b_all, f32).reshape(L, 4, 128).transpose(2, 0, 1).reshape(128, L * 4)

    def cols(v):  # (256,) -> (128, 2)
        return np.asarray(v, dtype=f32).reshape(2, 128).T

    bias_tbl = np.zeros((128, NB), dtype=f32)
    bias_tbl[:, COL_ENC0:COL_ENC0 + 2] = cols(inputs["enc_b0"])
    bias_tbl[:, COL_ENC1:COL_ENC1 + 2] = cols(inputs["enc_b1"])
    bias_tbl[:, COL_ENC2:COL_ENC2 + 2] = cols(inputs["enc_b2"])
    bias_tbl[:, COL_QB:COL_QB + 2] = cols(inputs["attn_q_b"])
    bias_tbl[:, COL_OB:COL_OB + 2] = cols(np.asarray(inputs["attn_o_b"], f32) * SQ2)
    bias_tbl[:, COL_DEC0:COL_DEC0 + 2] = cols(inputs["dec_b0"])
    bias_tbl[:, COL_DEC1:COL_DEC1 + 2] = cols(inputs["dec_b1"])
    bias_tbl[:, COL_DEC2:COL_DEC2 + 2] = cols(inputs["dec_b2"])
    bias_tbl[:, COL_DEC3:COL_DEC3 + 2] = cols(inputs["dec_b3"])
    bias2 = np.ascontiguousarray(np.concatenate([bias_tbl, hw_b], axis=1))

    shared = dict(hw_w=hw_w, bias2=bias2, wenc0p=wenc0p, pack1=pack1)

    in_maps = []
    for i in range(N_CORES):
        sl = slice(i * BS, (i + 1) * BS)
        kv_blocks = []
        for s in range(BS):
            for c in range(2):
                kv_blocks.append(keysT[i * BS + s, c * 128 : (c + 1) * 128, :])
        for s in range(BS):
            for c in range(2):
                kv_blocks.append(values[i * BS + s, c * 128 : (c + 1) * 128, :])
        wpack = npm(np.concatenate(
            [prefix] + kv_blocks[:4] + kv_blocks[4:] + [ones_blk, blast_blk], axis=1))
        m = dict(shared)
        m["x0"] = np.ascontiguousarray(x0[sl])
        m["wpack"] = wpack
        in_maps.append(m)
    return in_maps


def kernel(**inputs):
    global LAST_EXEC_NS
    if "nc" not in _BUILD_CACHE:
        _BUILD_CACHE["nc"] = _build()
    nc = _BUILD_CACHE["nc"]

    in_maps = _prep_host(inputs)

    trace = os.environ.get("KBENCH_TRACE", "0") == "1"
    if trace:
        _install_ntff_hook()
    res = run_bass_kernel_spmd(nc, in_maps, core_ids=list(range(N_CORES)), trace=trace)
    LAST_EXEC_NS = res.exec_time_ns

    mel = np.concatenate([r["mel"] for r in res.results], axis=0)
    attn = np.concatenate([r["attn"] for r in res.results], axis=0)
    done = np.concatenate([r["done"] for r in res.results], axis=0)
    return mel, attn, done


def _install_ntff_hook():
    """Register the axon NTFF profiling hook (missing from this image's antenv)."""
    import types

    if "antenv.axon_hooks" in sys.modules:
        return
    m = types.ModuleType("antenv.axon_hooks")
    m._h = None
    m.set_axon_ntff_profile_hook = lambda h: setattr(m, "_h", h)
    m.get_axon_ntff_profile_hook = lambda: m._h
    sys.modules["antenv.axon_hooks"] = m
    try:
        import antenv

        antenv.axon_hooks = m
        from trn_agent_boot.trn_boot import _ntff_profile_via_ctypes

        m._h = _ntff_profile_via_ctypes("/opt/axon/libaxon_pjrt.so")
    except Exception:
        m._h = None


# revision 29
# speedup vs baseline: 1.0017x; 1.0017x over previous
"""Trainium2 Bass kernel for nn_Decoder_42417097016016 (DCTTS-style decoder).

Sharding: pure data parallel over batch. B=16 samples -> 8 NeuronCores x 2
samples each; all weights replicated per core.

Layout: activations live on-chip as (channels, time) so every conv1x1 /
causal conv is a PE matmul with channels on partitions.  Causal convs with
dilation d are 3 accumulating matmuls per output tile with column shifts
(0, d, 2d) - left zero-padding falls out of the shifted PSUM accumulation.

The attention block computes scores (t,s), softmax along free dim (ACT Exp
with accum_out row sums), writes the normalized attn output directly, and
PE-transposes it to (s,t) for the context matmul.  mel and done share one
final matmul by concatenating fc_w as a 401st output column (padded to 402).

Matmul precision knob KBENCH_MM: "f16" (default, full PE rate), "f32r"
(reduced-precision fp32, half rate, needs even matmul geometry -> shifted x2
copies for odd-dilation taps), "f32" (exact, quarter rate).
"""

import math
import os
import sys

import numpy as np

for _p in ("/opt/trn_rl_repo", "/root/.axon_site/_ro/trn_rl_repo"):
    if os.path.isdir(_p) and _p not in sys.path:
        sys.path.append(_p)

import concourse.bass as bass
import concourse.tile as tile
from concourse import bacc, mybir
from concourse.bass_utils import run_bass_kernel_spmd

AF = mybir.ActivationFunctionType
ALU = mybir.AluOpType
AX = mybir.AxisListType
F32 = mybir.dt.float32
F32R = mybir.dt.float32r
F16 = mybir.dt.float16

N_CORES = 8
B, T, TE, D, F = 16, 512, 256, 256, 400
BS = B // N_CORES  # samples per core
DIL = [1, 3, 9, 27, 1, 3, 9, 27, 3, 3] + [1, 3, 9, 27, 1, 1]
L = len(DIL)  # 16 highway layers (10 encoder + 6 decoder)
SQ2 = math.sqrt(0.5)

MM_DT = os.environ.get("KBENCH_MM", "f16")

# bias table column assignments
COL_ENC0, COL_ENC1, COL_ENC2 = 0, 2, 4
COL_QB, COL_OB = 6, 8
COL_DEC0, COL_DEC1, COL_DEC2, COL_DEC3 = 10, 12, 14, 16
NB = 18

LAST_EXEC_NS = None
_BUILD_CACHE = {}


def _mm(nc, out, lhsT, rhs, **kw):
    nc.tensor.matmul(out, lhsT, rhs, **kw)


def _build():
    """Build the per-core Bass program (same program on all 8 cores)."""
    from concourse.masks import make_identity

    nc = bacc.Bacc("TRN2", target_bir_lowering=False, debug=False)
    dt = F32
    dtm = {"f16": F16, "f32r": F32R, "f32": F32}[MM_DT]
    use_x2 = MM_DT == "f32r"  # f32r matmuls need even offsets/N

    # ---- DRAM I/O (per-core shard shapes) ----
    # x0: (s, p, c, t) pre-chunked+padded on host -> one DMA per sample
    d_x0 = nc.dram_tensor("x0", [BS, 128, 4, T], dtm, kind="ExternalInput").ap()
    d_hw_w = nc.dram_tensor("hw_w", [L, 128, 2, 3, 4, 128], dtm, kind="ExternalInput").ap()
    # wenc0p: enc_w0^T padded 400->512 rows, chunked (128, 4, 256)
    d_wenc0 = nc.dram_tensor("wenc0p", [128, 4, D], dtm, kind="ExternalInput").ap()
    # pack1: [enc1 c0|c1, enc2 c0|c1] as (128, 4*256)
    d_pack1 = nc.dram_tensor("pack1", [128, 4 * D], dtm, kind="ExternalInput").ap()
    # bias2: [bias_tbl (18) | hw_b (L*4)] fp32
    d_bias2 = nc.dram_tensor("bias2", [128, NB + L * 4], dt, kind="ExternalInput").ap()
    # wpack: all attention/decoder weights + keys/values + ones/blast rows
    WQ_O, WO_O = 0, 512
    WD0_O, WD1_O, WD2_O, WD3_O = 1024, 2048, 2560, 3072
    WLAST_O = 3584
    KT_O = WLAST_O + 2 * (F + 2)          # 4388
    V_O = KT_O + 4 * TE                   # 5412
    ONES_O = V_O + 4 * D                  # 6436
    BLAST_O = ONES_O + 128                # 6564
    WPACK_COLS = BLAST_O + (F + 2)        # 6966
    d_wpack = nc.dram_tensor("wpack", [128, WPACK_COLS], dtm, kind="ExternalInput").ap()

    d_mel = nc.dram_tensor("mel", [BS, T, F], dt, kind="ExternalOutput").ap()
    d_attn = nc.dram_tensor("attn", [BS, T, TE], dt, kind="ExternalOutput").ap()
    d_done = nc.dram_tensor("done", [BS, T, 1], dt, kind="ExternalOutput").ap()

    with tile.TileContext(nc) as tc:
        with (
            tc.tile_pool(name="const", bufs=1) as const,
            tc.tile_pool(name="xpool", bufs=1) as xpool,
            tc.tile_pool(name="persist", bufs=1) as persist,
            tc.tile_pool(name="wstream", bufs=1) as wstream,
            tc.tile_pool(name="temp", bufs=1) as temp,
            tc.tile_pool(name="psum", bufs=1, space="PSUM") as psum,
        ):
            # ---------- startup-critical loads first (DMA queue is FIFO) ----
            wenc0_sb = const.tile([128, 4, D], dtm, name="wenc0_sb")
            nc.sync.dma_start(wenc0_sb, d_wenc0)
            w_enc0_sb = [wenc0_sb[:, c, :] for c in range(4)]
            xin = {}
            for s in range(BS):
                xt = temp.tile([128, 4, T], dtm, tag=f"xin_{s}", name=f"xin_{s}")
                nc.sync.dma_start(xt, d_x0[s])
                xin[s] = [xt[:, c, :] for c in range(4)]
            pack1_sb = const.tile([128, 4 * D], dtm, name="pack1_sb")
            nc.sync.dma_start(pack1_sb, d_pack1)
            w_enc1_sb = [pack1_sb[:, 0:D], pack1_sb[:, D : 2 * D]]
            w_enc2_sb = [pack1_sb[:, 2 * D : 3 * D], pack1_sb[:, 3 * D : 4 * D]]
            bias2_sb = const.tile([128, NB + L * 4], dt, name="bias2_sb")
            nc.sync.dma_start(bias2_sb, d_bias2)
            bias_sb = bias2_sb[:, 0:NB]

            def hwb_ap(l, j):
                c = NB + l * 4 + j
                return bias2_sb[:, c : c + 1]

            ident = const.tile([128, 128], dt, name="ident")
            make_identity(nc, ident)
            stat_sb = const.tile([128, 32], F32, name="stat_sb")
            # HAM warm-up: keep the PE busy during the input DMA wait so the
            # clock gate is already at 8/8 when real matmuls arrive.
            warm_ps = psum.tile([128, 128], F32, tag="bank7", name="warm_ps")
            for _w in range(24):
                nc.tensor.matmul(warm_ps, ident, ident, is_transpose=True,
                                 start=True, stop=True, skip_group_check=True)

            # helper: one conv1x1 block (256 outputs) for all samples.
            # Epilogue on DVE: out = relu?(psum + bias)
            def conv_block(w_tiles, rhs_per_s, relu, bias_col, banks, out_pool,
                           tag_fn, bufs=1, uid=""):
                outs = {s: [None, None] for s in range(BS)}
                pss = {}
                nk = len(w_tiles)
                for mt in range(2):
                    for s in range(BS):
                        pss[(s, mt)] = psum.tile(
                            [128, T], F32,
                            tag=f"bank{banks[s * 2 + mt]}",
                            name=f"ps_{uid}_{s}_{mt}",
                        )
                # chunk-major: all chunk-0 matmuls first so the producer of the
                # later chunks gets a full bank-sweep of runway
                for c in range(nk):
                    for mt in range(2):
                        for s in range(BS):
                            _mm(nc, pss[(s, mt)], w_tiles[c][:, mt * 128 : (mt + 1) * 128],
                                rhs_per_s[s][c], start=(c == 0), stop=(c == nk - 1))
                for mt in range(2):
                    for s in range(BS):
                        ot = out_pool.tile(
                            [128, T], dtm,
                            tag=tag_fn(s, mt), bufs=bufs,
                            name=f"{uid}_{s}_{mt}",
                        )
                        b_ap = bias_sb[:, bias_col + mt : bias_col + mt + 1]
                        if mt == 0:  # split epilogues across DVE and ACT
                            if relu:
                                nc.vector.tensor_scalar(ot, pss[(s, mt)], b_ap, 0.0,
                                                        op0=ALU.add, op1=ALU.max)
                            else:
                                nc.vector.tensor_scalar_add(ot, pss[(s, mt)], b_ap)
                        else:
                            nc.scalar.activation(
                                ot, pss[(s, mt)], AF.Relu if relu else AF.Identity,
                                bias=b_ap, scale=1.0)
                        outs[s][mt] = ot
                return outs

            xtag = lambda s, mt: f"x_{s}_{mt}"
            gatag = lambda s, mt: f"ga_{s}_{mt}"
            gbtag = lambda s, mt: f"gb_{s}_{mt}"

            # ---------- encoder head ----------
            h1 = conv_block(w_enc0_sb, xin, True, COL_ENC0, [0, 1, 4, 5], temp, gatag, uid="h1")
            h2 = conv_block(w_enc1_sb, h1, True, COL_ENC1, [2, 3, 6, 7], temp, gbtag, uid="h2")
            xs = conv_block(w_enc2_sb, h2, False, COL_ENC2, [0, 1, 4, 5], xpool, xtag,
                            bufs=4, uid="xe")

            # remaining const loads are deferred (emitted mid-encoder so the
            # first highway-layer weight DMAs aren't stuck behind them)
            misc = {}

            def load_misc_consts():
                wp = const.tile([128, WPACK_COLS], dtm, name="wpack_sb")
                nc.sync.dma_start(wp, d_wpack)
                misc["w_q"] = [wp[:, WQ_O : WQ_O + D], wp[:, WQ_O + D : WQ_O + 2 * D]]
                misc["w_o"] = [wp[:, WO_O : WO_O + D], wp[:, WO_O + D : WO_O + 2 * D]]
                misc["w_dec0"] = [wp[:, WD0_O + c * D : WD0_O + (c + 1) * D] for c in range(4)]
                misc["w_dec1"] = [wp[:, WD1_O : WD1_O + D], wp[:, WD1_O + D : WD1_O + 2 * D]]
                misc["w_dec2"] = [wp[:, WD2_O : WD2_O + D], wp[:, WD2_O + D : WD2_O + 2 * D]]
                misc["w_dec3"] = [wp[:, WD3_O : WD3_O + D], wp[:, WD3_O + D : WD3_O + 2 * D]]
                misc["w_last"] = [wp[:, WLAST_O : WLAST_O + F + 2],
                                  wp[:, WLAST_O + F + 2 : WLAST_O + 2 * (F + 2)]]
                misc["ones"] = wp[0:2, ONES_O : ONES_O + 128]
                misc["blast"] = wp[0:2, BLAST_O : BLAST_O + F + 2]
                kT_sb, v_sb = {}, {}
                for s in range(BS):
                    for c in range(2):
                        i = s * 2 + c
                        kT_sb[(s, c)] = wp[:, KT_O + i * TE : KT_O + (i + 1) * TE]
                        v_sb[(s, c)] = wp[:, V_O + i * D : V_O + (i + 1) * D]
                misc["kT"] = kT_sb
                misc["v"] = v_sb

            # ---------- highway stack ----------
            def make_x2(xs_cur, uid):
                # right-shifted copy (col0 = 0): makes the odd-dilation middle
                # conv tap even-aligned for the f32r matmul mode.
                out = {s: [None, None] for s in range(BS)}
                for c in range(2):
                    for s in range(BS):
                        x2 = xpool.tile([128, T], dtm, tag=f"x2_{s}_{c}", bufs=2,
                                        name=f"x2_{uid}_{s}_{c}")
                        nc.gpsimd.tensor_scalar_mul(x2[:, 0:1], xs_cur[s][c][:, 0:1], 0.0)
                        nc.gpsimd.tensor_copy(x2[:, 1:T], xs_cur[s][c][:, 0 : T - 1])
                        out[s][c] = x2
                return out

            def highway_layers(l_lo, l_hi, x2s):
                nonlocal xs
                for l in range(l_lo, l_hi):
                    dil = DIL[l]
                    wt = wstream.tile([128, 2, 3, 4, 128], dtm, tag="hww", bufs=3,
                                      name=f"hw_w_{l}")
                    nc.sync.dma_start(wt, d_hw_w[l])
                    last_enc = l == 9
                    ps_all = {}
                    for mt in range(4):
                        for s in range(BS):
                            ps_all[(mt, s)] = psum.tile(
                                [128, T], F32, tag=f"bank{mt * 2 + s}",
                                name=f"hwps_{l}_{mt}_{s}",
                            )
                    # kc-major: ALL kc0 taps (24 matmuls) before any kc1 tap,
                    # giving the previous layer's x'[1] epilogue a ~5us runway.
                    # mt order (2,0,3,1): chunk-0 gate+input banks finish first
                    # so its epilogue chain overlaps the remaining matmuls.
                    seen = {}
                    for kc in range(2):
                        for mt in (2, 0, 3, 1):
                            for k in (2, 1, 0):
                                first = kc == 0 and k == 2
                                last = kc == 1 and k == 0
                                for s in range(BS):
                                    ps = ps_all[(mt, s)]
                                    if k == 2:
                                        _mm(nc, ps, wt[:, kc, k, mt, :], xs[s][kc],
                                            start=first, stop=last)
                                    elif k == 1:
                                        if use_x2:
                                            off = dil - 1
                                            _mm(nc, ps[:, off:T], wt[:, kc, k, mt, :],
                                                x2s[s][kc][:, 0 : T - off],
                                                start=first, stop=last)
                                        else:
                                            _mm(nc, ps[:, dil:T], wt[:, kc, k, mt, :],
                                                xs[s][kc][:, 0 : T - dil],
                                                start=first, stop=last)
                                    else:
                                        sh = 2 * dil
                                        _mm(nc, ps[:, sh:T], wt[:, kc, k, mt, :],
                                            xs[s][kc][:, 0 : T - sh],
                                            start=first, stop=last)
                    # epilogue: x' = x + sigmoid(g) * (a - x)
                    # per chunk c: g = bank (c+2), a = bank c
                    newxs = {s: [None, None] for s in range(BS)}
                    for c in range(2):
                        tgs = {}
                        for s in range(BS):
                            tg = temp.tile([128, T], dtm, tag=f"tg_{s}_{c}", bufs=3,
                                           name=f"tg_{l}_{s}_{c}")
                            nc.scalar.activation(
                                tg, ps_all[(c + 2, s)], AF.Sigmoid,
                                bias=hwb_ap(l, c + 2), scale=1.0)
                            tgs[s] = tg
                        tmps = {}
                        for s in range(BS):
                            tmp = temp.tile([128, T], dtm, tag=f"tmp_{s}_{c}", bufs=3,
                                            name=f"tmp_{l}_{s}_{c}")
                            nc.vector.scalar_tensor_tensor(
                                tmp, ps_all[(c, s)], hwb_ap(l, c),
                                xs[s][c], op0=ALU.add, op1=ALU.subtract)
                            tmps[s] = tmp
                        for s in range(BS):
                            nc.vector.tensor_mul(tmps[s], tgs[s], tmps[s])
                        for s in range(BS):
                            if last_enc:
                                xn = persist.tile([128, T], dtm, tag=f"q_{s}_{c}",
                                                  name=f"Q_{s}_{c}")
                            else:
                                xn = xpool.tile([128, T], dtm, tag=f"x_{s}_{c}", bufs=4,
                                                name=f"x_{l + 1}_{s}_{c}")
                            nc.vector.tensor_add(xn, tmps[s], xs[s][c])
                            newxs[s][c] = xn
                    xs = newxs
                    if use_x2 and l + 1 < l_hi:
                        x2s = make_x2(xs, f"l{l + 1}")
                    if l == 1:
                        load_misc_consts()

            # ---------- encoder highway ----------
            highway_layers(0, 10, make_x2(xs, "e0") if use_x2 else None)
            w_q_sb, w_o_sb = misc["w_q"], misc["w_o"]
            w_dec0_sb, w_dec1_sb = misc["w_dec0"], misc["w_dec1"]
            w_dec2_sb, w_dec3_sb = misc["w_dec2"], misc["w_dec3"]
            w_last_sb, ones_row, blast_sb = misc["w_last"], misc["ones"], misc["blast"]
            kT_sb, v_sb = misc["kT"], misc["v"]
            Qs = xs  # persisted encoder output (D, T) tiles

            # ---------- attention ----------
            Rqs = {s: [] for s in range(BS)}
            q_sb = {s: [] for s in range(BS)}
            qpss = {}
            for mt in range(2):
                for s in range(BS):
                    qpss[(s, mt)] = psum.tile([128, T], F32, tag=f"bank{s * 4 + mt}",
                                              name=f"qps_{s}_{mt}")
            for kc in range(2):
                for mt in range(2):
                    for s in range(BS):
                        _mm(nc, qpss[(s, mt)], w_q_sb[kc][:, mt * 128 : (mt + 1) * 128],
                            Qs[s][kc], start=(kc == 0), stop=(kc == 1))
            for mt in range(2):
                for s in range(BS):
                    qt = temp.tile([128, T], dtm, tag=f"ga_{s}_{mt}", name=f"q_{s}_{mt}")
                    nc.vector.tensor_scalar_add(
                        qt, qpss[(s, mt)], bias_sb[:, COL_QB + mt : COL_QB + mt + 1])
                    q_sb[s].append(qt)

            # softmax without max-subtraction: scores are O(10), exp is safe in
            # fp32 and softmax is shift-invariant, so this matches the reference.
            # Stage-major across samples so the PE always has the other
            # sample's matmuls while one sample's softmax chain runs.
            at_tiles, psts, aT = {}, {}, {s: [] for s in range(BS)}
            for s in range(BS):
                at_tiles[s] = temp.tile([128, 4, TE], dt, tag=f"att_{s}",
                                        name=f"att_{s}")
                psts[s] = [psum.tile([128, T], F32, tag=f"bank{s * 4 + 2 + sc}",
                                     name=f"tps_{s}_{sc}") for sc in range(2)]
            for s in range(BS):
                for tt in range(4):
                    ps = psum.tile([128, TE], F32, tag=f"bank{s * 4 + (tt % 2)}",
                                   name=f"sps_{s}_{tt}")
                    for dc in range(2):
                        _mm(nc, ps, q_sb[s][dc][:, tt * 128 : (tt + 1) * 128],
                            kT_sb[(s, dc)], start=(dc == 0), stop=(dc == 1))
                    st = stat_sb[:, (s * 4 + tt) * 4 : (s * 4 + tt) * 4 + 4]
                    at = at_tiles[s][:, tt, :]
                    nc.scalar.activation(at, ps, AF.Exp, accum_out=st[:, 1:2])
                    nc.vector.reciprocal(st[:, 2:3], st[:, 1:2])
                    nc.vector.tensor_scalar_mul(at, at, st[:, 2:3])
            # dec conv0, part 1: Q-chunk matmuls are attention-independent ->
            # emit them here so the PE stays busy (and HAM warm) while the
            # softmax chains run.  Result bounces to SBUF; part 2 adds it back.
            qcontrib = {}
            d0aps = {}
            for s in range(BS):
                for mt in range(2):
                    d0aps[(s, mt)] = psum.tile([128, T], F32, tag=f"bank{s * 4 + mt}",
                                               name=f"ps_xd0a_{s}_{mt}")
            for c in range(2):
                for s in range(BS):
                    for mt in range(2):
                        _mm(nc, d0aps[(s, mt)], w_dec0_sb[c][:, mt * 128 : (mt + 1) * 128],
                            Qs[s][c], start=(c == 0), stop=(c == 1))
            for s in range(BS):
                for mt in range(2):
                    qc = temp.tile([128, T], F32, tag=f"qc_{s}_{mt}",
                                   name=f"qc_{s}_{mt}")
                    if mt == 0:
                        nc.vector.tensor_copy(qc, d0aps[(s, mt)])
                    else:
                        nc.scalar.copy(qc, d0aps[(s, mt)])
                    qcontrib[(s, mt)] = qc
            for s in range(BS):
                for tt in range(4):
                    at = at_tiles[s][:, tt, :]
                    for sc in range(2):
                        nc.tensor.matmul(
                            psts[s][sc][:, tt * 128 : (tt + 1) * 128],
                            at[:, sc * 128 : (sc + 1) * 128],
                            ident, is_transpose=True, start=True, stop=True,
                            skip_group_check=True)
                nc.sync.dma_start(
                    d_attn[s].rearrange("(tt p) e -> p tt e", p=128), at_tiles[s])
            for s in range(BS):
                for sc in range(2):
                    a2 = temp.tile([128, T], dtm, tag=f"gb_{s}_{sc}", name=f"aT_{s}_{sc}")
                    nc.vector.tensor_copy(a2, psts[s][sc])
                    aT[s].append(a2)

            ctx_sb = {s: [] for s in range(BS)}
            for s in range(BS):
                for dc in range(2):
                    ps = psum.tile([128, T], F32, tag=f"bank{s * 4 + dc}",
                                   name=f"cps_{s}_{dc}")
                    for sc in range(2):
                        _mm(nc, ps, v_sb[(s, sc)][:, dc * 128 : (dc + 1) * 128],
                            aT[s][sc], start=(sc == 0), stop=(sc == 1))
                    ct = temp.tile([128, T], dtm, tag=f"ga_{s}_{dc}", name=f"ctx_{s}_{dc}")
                    nc.vector.tensor_copy(ct, ps)
                    ctx_sb[s].append(ct)

            for mt in range(2):
                for s in range(BS):
                    ps = psum.tile([128, T], F32, tag=f"bank{s * 4 + mt}",
                                   name=f"ops_{s}_{mt}")
                    for dc in range(2):
                        _mm(nc, ps, w_o_sb[dc][:, mt * 128 : (mt + 1) * 128],
                            ctx_sb[s][dc], start=(dc == 0), stop=(dc == 1))
                    tmpo = temp.tile([128, T], dt, tag=f"gb_{s}_{mt}",
                                     name=f"tmpo_{s}_{mt}")
                    nc.vector.tensor_scalar_add(
                        tmpo, ps, bias_sb[:, COL_OB + mt : COL_OB + mt + 1])
                    rq = persist.tile([128, T], dtm, tag=f"rq_{s}_{mt}",
                                      name=f"rq_{s}_{mt}")
                    # Rq = sqrt(.5)*query + out_proj  [scales folded into w_o/b_o]
                    nc.vector.scalar_tensor_tensor(
                        rq, Qs[s][mt], SQ2, tmpo, op0=ALU.mult, op1=ALU.add)
                    Rqs[s].append(rq)

            # ---------- decoder (conv0 part 2: Rq chunks + fused epilogue) ----
            xs = {s: [None, None] for s in range(BS)}
            d0ps = {}
            for mt in range(2):
                for s in range(BS):
                    d0ps[(s, mt)] = psum.tile(
                        [128, T], F32, tag=f"bank{[2, 3, 6, 7][s * 2 + mt]}",
                        name=f"ps_xd0b_{s}_{mt}")
            for c in range(2):
                for mt in range(2):
                    for s in range(BS):
                        _mm(nc, d0ps[(s, mt)], w_dec0_sb[2 + c][:, mt * 128 : (mt + 1) * 128],
                            Rqs[s][c], start=(c == 0), stop=(c == 1))
            for mt in range(2):
                for s in range(BS):
                    ot = xpool.tile([128, T], dtm, tag=f"x_{s}_{mt}", bufs=4,
                                    name=f"xd0_{s}_{mt}")
                    b_ap = bias_sb[:, COL_DEC0 + mt : COL_DEC0 + mt + 1]
                    nc.vector.scalar_tensor_tensor(
                        ot, d0ps[(s, mt)], b_ap, qcontrib[(s, mt)],
                        op0=ALU.add, op1=ALU.add)
                    xs[s][mt] = ot
            highway_layers(10, 16, make_x2(xs, "d0") if use_x2 else None)
            xs = conv_block(w_dec1_sb, xs, True, COL_DEC1, [0, 1, 4, 5], xpool, xtag,
                            bufs=4, uid="xd1")
            xs = conv_block(w_dec2_sb, xs, True, COL_DEC2, [2, 3, 6, 7], xpool, xtag,
                            bufs=4, uid="xd2")
            xs = conv_block(w_dec3_sb, xs, True, COL_DEC3, [0, 1, 4, 5], xpool, xtag,
                            bufs=4, uid="xd3")

            # ---------- final: mel (per-tt sigmoid conv) + done ((1,T) row) ----
            for s in range(BS):
                # done = sigmoid(fc . x) computed as a single-row matmul so the
                # output DMA is one contiguous 2KB write (not a 512-desc scatter)
                psd = psum.tile([1, T], F32, tag=f"bank{s * 4 + 3}", name=f"dps_{s}")
                for dc in range(2):
                    _mm(nc, psd, w_last_sb[dc][:, F : F + 1], xs[s][dc],
                        start=(dc == 0), stop=(dc == 1))
                dn = temp.tile([1, T], dt, tag=f"done_{s}", name=f"done_{s}")
                nc.scalar.activation(dn, psd, AF.Sigmoid, scale=1.0,
                                     bias=blast_sb[0:1, F : F + 1])
                nc.sync.dma_start(d_done[s].rearrange("t o -> o t"), dn)
            for s in range(BS):
                fo = temp.tile([128, 4, F + 2], dt, tag=f"fin_{s}", name=f"fin_{s}")
                for tt in range(4):
                    ps = psum.tile([128, F + 2], F32, tag=f"bank{s * 4 + tt}",
                                   name=f"fps_{s}_{tt}")
                    for dc in range(2):
                        _mm(nc, ps, xs[s][dc][:, tt * 128 : (tt + 1) * 128],
                            w_last_sb[dc], start=(dc == 0), stop=False)
                    _mm(nc, ps, ones_row, blast_sb, start=False, stop=True)
                    nc.scalar.activation(fo[:, tt, :], ps, AF.Sigmoid, scale=1.0)
                    nc.sync.dma_start(d_mel[s, tt * 128 : (tt + 1) * 128, :],
                                      fo[:, tt, 0:F])

    nc.compile()
    return nc


def _prep_host(inputs):
    """Host-side packing: transposes, chunking, and packed const blocks."""
    f32 = np.float32
    mm_np = np.float16 if MM_DT == "f16" else np.float32

    def npm(a):
        return np.ascontiguousarray(np.asarray(a, dtype=f32)).astype(mm_np)

    # x0: (B, T, F) -> pad F to 512 -> (B, 128, 4, T)
    x_t = np.zeros((B, 512, T), f32)
    x_t[:, :F, :] = np.asarray(inputs["inputs"], f32).transpose(0, 2, 1)
    x0 = npm(x_t.reshape(B, 4, 128, T).transpose(0, 2, 1, 3))

    keysT = np.asarray(inputs["keys"], f32).transpose(0, 2, 1)  # (B, D, TE)
    values = np.asarray(inputs["values"], f32)  # (B, TE, D)

    w_all = np.concatenate([np.asarray(inputs["enc_hw_w"]),
                            np.asarray(inputs["dec_hw_w"])], axis=0)  # (16, 512, 256, 3)
    wt = w_all.transpose(0, 2, 1, 3)            # (L, ci, co, k)
    wt = wt.reshape(L, 2, 128, 4, 128, 3)       # (L, kc, p, mt, f, k)
    hw_w = npm(wt.transpose(0, 2, 1, 5, 3, 4))  # (L, 128, kc, k, mt, f)

    def t2(w):  # (O, I, 1) -> (I, O) fp32
        return np.asarray(w, f32)[:, :, 0].T

    # wenc0p: (400, 256) -> pad rows to 512 -> (128, 4, 256)
    we0 = np.zeros((512, D), f32)
    we0[:F] = t2(inputs["enc_w0"])
    wenc0p = npm(we0.reshape(4, 128, D).transpose(1, 0, 2))

    def chunks(w):  # (rows, cols) -> list of (128, cols)
        return [w[c * 128 : (c + 1) * 128] for c in range(w.shape[0] // 128)]

    pack1 = npm(np.concatenate(
        chunks(t2(inputs["enc_w1"])) + chunks(t2(inputs["enc_w2"])), axis=1))

    # wpack: [wq | wo | wdec0 | wdec1 | wdec2 | wdec3 | wlast | kT | v | ones | blast]
    w_q = np.asarray(inputs["attn_q_w"], f32).T
    w_o = np.asarray(inputs["attn_o_w"], f32).T * (math.sqrt(TE) * SQ2)
    w_last = np.concatenate(
        [np.asarray(inputs["last_w"], f32)[:, :, 0].T,
         np.asarray(inputs["fc_w"], f32).T,
         np.zeros((D, 1), f32)], axis=1)  # (256, 402)
    wd0 = chunks(t2(inputs["dec_w0"]))
    wd0 = [wd0[2], wd0[3], wd0[0], wd0[1]]  # ci chunks reordered: Q first, Rq last
    blocks = (chunks(w_q) + chunks(w_o) + wd0
              + chunks(t2(inputs["dec_w1"])) + chunks(t2(inputs["dec_w2"]))
              + chunks(t2(inputs["dec_w3"])) + chunks(w_last))
    # keys/values are per-core; build the shared prefix once
    prefix = np.concatenate(blocks, axis=1)  # (128, 4388)
    ones_blk = np.zeros((128, 128), f32)
    ones_blk[0, :] = 1.0
    blast_blk = np.zeros((128, F + 2), f32)
    blast_blk[0, :F] = np.asarray(inputs["last_b"], f32)
    blast_blk[0, F] = np.asarray(inputs["fc_b"], f32)[0]

    b_all = np.concatenate([np.asarray(inputs["enc_hw_b"]),
                            np.asarray(inputs["dec_hw_b"])], axis=0)  # (16, 512)
    hw_b = np.asarray(b_all, f32).reshape(L, 4, 128).transpose(2, 0, 1).reshape(128, L * 4)

    def cols(v):  # (256,) -> (128, 2)
        return np.asarray(v, dtype=f32).reshape(2, 128).T

    bias_tbl = np.zeros((128, NB), dtype=f32)
    bias_tbl[:, COL_ENC0:COL_ENC0 + 2] = cols(inputs["enc_b0"])
    bias_tbl[:, COL_ENC1:COL_ENC1 + 2] = cols(inputs["enc_b1"])
    bias_tbl[:, COL_ENC2:COL_ENC2 + 2] = cols(inputs["enc_b2"])
    bias_tbl[:, COL_QB:COL_QB + 2] = cols(inputs["attn_q_b"])
    bias_tbl[:, COL_OB:COL_OB + 2] = cols(np.asarray(inputs["attn_o_b"], f32) * SQ2)
    bias_tbl[:, COL_DEC0:COL_DEC0 + 2] = cols(inputs["dec_b0"])
    bias_tbl[:, COL_DEC1:COL_DEC1 + 2] = cols(inputs["dec_b1"])
    bias_tbl[:, COL_DEC2:COL_DEC2 + 2] = cols(inputs["dec_b2"])
    bias_tbl[:, COL_DEC3:COL_DEC3 + 2] = cols(inputs["dec_b3"])
    bias2 = np.ascontiguousarray(np.concatenate([bias_tbl, hw_b], axis=1))

    shared = dict(hw_w=hw_w, bias2=bias2, wenc0p=wenc0p, pack1=pack1)

    in_maps = []
    for i in range(N_CORES):
        sl = slice(i * BS, (i + 1) * BS)
        kv_blocks = []
        for s in range(BS):
            for c in range(2):
                kv_blocks.append(keysT[i * BS + s, c * 128 : (c + 1) * 128, :])
        for s in range(BS):
            for c in range(2):
                kv_blocks.append(values[i * BS + s, c * 128 : (c + 1) * 128, :])
        wpack = npm(np.concatenate(
            [prefix] + kv_blocks[:4] + kv_blocks[4:] + [ones_blk, blast_blk], axis=1))
        m = dict(shared)
        m["x0"] = np.ascontiguousarray(x0[sl])
        m["wpack"] = wpack
        in_maps.append(m)
    return in_maps


def kernel(**inputs):
    global LAST_EXEC_NS
    if "nc" not in _BUILD_CACHE:
        _BUILD_CACHE["nc"] = _build()
    nc = _BUILD_CACHE["nc"]

    in_maps = _prep_host(inputs)

    trace = os.environ.get("KBENCH_TRACE", "0") == "1"
    if trace:
        _install_ntff_hook()
    res = run_bass_kernel_spmd(nc, in_maps, core_ids=list(range(N_CORES)), trace=trace)
    LAST_EXEC_NS = res.exec_time_ns

    mel = np.concatenate([r["mel"] for r in res.results], axis=0)
    attn = np.concatenate([r["attn"] for r in res.results], axis=0)
    done = np.concatenate([r["done"] for r in res.results], axis=0)
    return mel, attn, done


def _install_ntff_hook():
    """Register the axon NTFF profiling hook (missing from this image's antenv)."""
    import types

    if "antenv.axon_hooks" in sys.modules:
        return
    m = types.ModuleType("antenv.axon_hooks")
    m._h = None
    m.set_axon_ntff_profile_hook = lambda h: setattr(m, "_h", h)
    m.get_axon_ntff_profile_hook = lambda: m._h
    sys.modules["antenv.axon_hooks"] = m
    try:
        import antenv

        antenv.axon_hooks = m
        from trn_agent_boot.trn_boot import _ntff_profile_via_ctypes

        m._h = _ntff_profile_via_ctypes("/opt/axon/libaxon_pjrt.so")
    except Exception:
        m._h = None
